# revision 1
# baseline (speedup 1.0000x reference)
"""Trainium2 Bass kernel for nn_Loss_factory_12429635355015.

Loss = NLLSurv + CohortLoss(intra + inter) over a [4, 8192, 4, 256] cohort bank.

Strategy (memory-bound, 8 NeuronCores):
  - Shard cohort_bank along the N (bank-entry) axis: each core streams its
    16 MiB shard once at HBM line rate (8 tiles x 2 MiB contiguous DMAs).
  - Per 512-entry tile (4 entries per partition):
      DVE:  component-sum over the 4 bank components  (S = sum_j bank[:,j,:])
      ACT:  Square+accum_out -> per-entry sum-of-squares; rsqrt via exp(-.5 ln x)
      PE :  2x transpose S -> [c, n] chunks; matmul vs l2-normalized anchors
      ACT:  e = exp(sims * (0.5/||S||))   (per-partition scale fuses l2norm+tau)
      PE :  ones-matmul accumulates per-class per-sample sums in PSUM
  - NLL + intra terms are computed on-device from host-encoded one-hots
    (index encoding only; all arithmetic on device).
  - Each core outputs [ep_partial, en_partial, nll+intra]; the host sums the
    two scalars across cores (the 'all-reduce two scalars' step) and applies
    the final -log((ep+eps)/(ep+en+eps)).
"""

import math
import os
import sys

import numpy as np

for _p in ("/opt/trn_rl_repo",):
    if _p not in sys.path and os.path.isdir(_p):
        sys.path.insert(0, _p)

import concourse.bacc as bacc
import concourse.tile as tile
from concourse import mybir
from concourse.bass_utils import run_bass_kernel_spmd

# Pin every activation to the one table set that contains all functions this
# kernel uses (Square/Ln/Exp/Copy/Abs/Identity). Without this, Bacc's
# first-match set selection alternates between sets (Ln lives outside the
# default exp set) and reloads the ACT tables ~1.3us per switch every tile.
_ACT_SET = "natural_log_exp_and_others"


def _pin_act_tables():
    import functools
    import concourse.hw_specs as hw_specs
    if getattr(hw_specs.get_activation_tables, "_pinned", False):
        return
    orig = hw_specs.get_activation_tables

    @functools.cache
    def pinned(arch):
        tabs = orig(arch)
        return {k: (v if k == _ACT_SET else set()) for k, v in tabs.items()}

    pinned._pinned = True
    hw_specs.get_activation_tables = pinned
    bacc.get_activation_tables = pinned


_pin_act_tables()

F32 = mybir.dt.float32
AF = mybir.ActivationFunctionType

# Problem constants (hardcoded per spec).
B = 64            # batch
K = 4             # n_cls
C = 256           # feature dim
NB = 8192         # bank entries per class (global)
NCORES = 8
NSH = NB // NCORES          # 1024 bank entries per class per core
ROWS = K * NSH              # 4096 rows of [4*256] per core
NT = 512                    # bank entries per tile (2 MiB)
TILES = ROWS // NT          # 8
TILES_PER_CLASS = NSH // NT # 2
EPG = NT // 128             # 4 entries per partition per tile
LN_HALF = math.log(0.5)
EPS_NLL = 1e-7
EPS_COH = 1e-8


def _build():
    nc = bacc.Bacc("TRN2", target_bir_lowering=False, debug=False,
                   enable_asserts=False, num_devices=NCORES)

    bank = nc.dram_tensor("bank", [ROWS, 1024], F32, kind="ExternalInput")
    indiv = nc.dram_tensor("indiv", [B, 1024], F32, kind="ExternalInput")
    gp = nc.dram_tensor("gp", [B, 512], F32, kind="ExternalInput")
    haz = nc.dram_tensor("haz", [B, K], F32, kind="ExternalInput")
    spad = nc.dram_tensor("spad", [B, K + 1], F32, kind="ExternalInput")
    ohy = nc.dram_tensor("ohy", [B, K + 1], F32, kind="ExternalInput")
    ohy1 = nc.dram_tensor("ohy1", [B, K + 1], F32, kind="ExternalInput")
    oh4 = nc.dram_tensor("oh4", [B, K], F32, kind="ExternalInput")
    cfs = nc.dram_tensor("cfs", [B, 2], F32, kind="ExternalInput")

    out_d = nc.dram_tensor("out_vec", [4, 1], F32, kind="ExternalOutput")
    e_dbg = nc.dram_tensor("e_dbg", [B, K], F32, kind="ExternalOutput")

    import ml_dtypes
    ident_d = nc.inline_tensor(np.eye(128, dtype=np.float32), "ident")
    ident_bf_d = nc.inline_tensor(np.eye(128, dtype=ml_dtypes.bfloat16), "ident_bf")
    ones_d = nc.inline_tensor(np.ones((128, 1), dtype=np.float32), "ones_col")

    v = nc.vector
    a = nc.scalar

    with tile.TileContext(nc) as tc:
        from contextlib import ExitStack
        with ExitStack() as ctx:
            const = ctx.enter_context(tc.tile_pool(name="const", bufs=1))
            small = ctx.enter_context(tc.tile_pool(name="small", bufs=1))
            tpool = ctx.enter_context(tc.tile_pool(name="T", bufs=3))
            spool = ctx.enter_context(tc.tile_pool(name="S", bufs=2))
            stpool = ctx.enter_context(tc.tile_pool(name="STsb", bufs=3))
            epool = ctx.enter_context(tc.tile_pool(name="esb", bufs=3))
            sqpool = ctx.enter_context(tc.tile_pool(name="sq", bufs=2))
            ps_st = ctx.enter_context(tc.tile_pool(name="ps_st", bufs=2, space="PSUM"))
            ps_p = ctx.enter_context(tc.tile_pool(name="ps_p", bufs=2, space="PSUM"))
            ps_one = ctx.enter_context(tc.tile_pool(name="ps_one", bufs=1, space="PSUM"))

            BF16 = mybir.dt.bfloat16
            ident_sb = const.tile([128, 128], F32)
            nc.sync.dma_start(out=ident_sb[:], in_=ident_d[:])
            ident_bf = const.tile([128, 128], BF16)
            nc.sync.dma_start(out=ident_bf[:], in_=ident_bf_d[:])
            ones_sb = const.tile([128, 1], F32)
            nc.sync.dma_start(out=ones_sb[:], in_=ones_d[:])

            # ---------- anchors: A = l2norm(mean_j indiv[b,j,:]) ----------
            ind_sb = small.tile([B, 1024], F32)
            nc.sync.dma_start(out=ind_sb[:], in_=indiv[:])
            iv = ind_sb.rearrange("p (j c) -> p j c", j=4)
            asum = small.tile([B, C], F32)
            atmp = small.tile([B, C], F32)
            v.tensor_add(asum[:], iv[:, 0, :], iv[:, 1, :])
            v.tensor_add(atmp[:], iv[:, 2, :], iv[:, 3, :])
            v.tensor_add(asum[:], asum[:], atmp[:])
            sqa = small.tile([B, C], F32)
            ssa = small.tile([B, 1], F32)
            a.activation(sqa[:], asum[:], AF.Square, accum_out=ssa[:])
            lna = small.tile([B, 1], F32)
            a.activation(lna[:], ssa[:], AF.Ln)
            rsa = small.tile([B, 1], F32)
            a.activation(rsa[:], lna[:], AF.Exp, scale=-0.5)
            v.tensor_scalar_mul(asum[:], asum[:], rsa[:])
            at_ps = ps_one.tile([128, 2, B], F32, tag="at")
            for h in range(2):
                nc.tensor.transpose(at_ps[:, h, :], asum[:, h * 128:(h + 1) * 128],
                                    ident_sb[0:B, 0:B])
            at_sb = const.tile([128, 2, B], BF16)
            a.copy(at_sb[:], at_ps[:])

            # ---------- NLL (per-b, b on partitions) ----------
            haz_sb = small.tile([B, K], F32)
            nc.sync.dma_start(out=haz_sb[:], in_=haz[:])
            spad_sb = small.tile([B, K + 1], F32)
            nc.sync.dma_start(out=spad_sb[:], in_=spad[:])
            ohy_sb = small.tile([B, K + 1], F32)
            nc.sync.dma_start(out=ohy_sb[:], in_=ohy[:])
            ohy1_sb = small.tile([B, K + 1], F32)
            nc.sync.dma_start(out=ohy1_sb[:], in_=ohy1[:])
            oh4_sb = small.tile([B, K], F32)
            nc.sync.dma_start(out=oh4_sb[:], in_=oh4[:])
            cfs_sb = small.tile([B, 2], F32)
            nc.sync.dma_start(out=cfs_sb[:], in_=cfs[:])

            t5 = small.tile([B, K + 1], F32)
            t4 = small.tile([B, K], F32)
            sy = small.tile([B, 1], F32)
            hy = small.tile([B, 1], F32)
            sy1 = small.tile([B, 1], F32)
            v.tensor_mul(t5[:], spad_sb[:], ohy_sb[:])
            v.reduce_sum(sy[:], t5[:], axis=mybir.AxisListType.X)
            v.tensor_mul(t4[:], haz_sb[:], ohy_sb[:, 0:K])
            v.reduce_sum(hy[:], t4[:], axis=mybir.AxisListType.X)
            v.tensor_mul(t5[:], spad_sb[:], ohy1_sb[:])
            v.reduce_sum(sy1[:], t5[:], axis=mybir.AxisListType.X)
            for x in (sy, hy, sy1):
                v.tensor_scalar_max(x[:], x[:], EPS_NLL)
            lsy = small.tile([B, 1], F32)
            lhy = small.tile([B, 1], F32)
            lsy1 = small.tile([B, 1], F32)
            a.activation(lsy[:], sy[:], AF.Ln)
            a.activation(lhy[:], hy[:], AF.Ln)
            a.activation(lsy1[:], sy1[:], AF.Ln)
            tu = small.tile([B, 1], F32)
            tcen = small.tile([B, 1], F32)
            negl = small.tile([B, 1], F32)
            v.tensor_add(tu[:], lsy[:], lhy[:])
            v.tensor_mul(tu[:], tu[:], cfs_sb[:, 1:2])      # *(1-cf)
            v.tensor_mul(tcen[:], lsy1[:], cfs_sb[:, 0:1])  # *cf
            v.tensor_add(negl[:], tu[:], tcen[:])           # = -neg_l per b

            # ---------- intra cohort term ----------
            gp_sb = small.tile([B, 512], F32)
            nc.sync.dma_start(out=gp_sb[:], in_=gp[:])
            sqi = small.tile([B, 1024], F32)
            v.tensor_mul(sqi[:], ind_sb[:], ind_sb[:])
            ssqi = small.tile([B, 4], F32)
            v.reduce_sum(ssqi[:], sqi.rearrange("p (j c) -> p j c", j=4),
                         axis=mybir.AxisListType.X)
            rsi = small.tile([B, 4], F32)
            a.activation(rsi[:], ssqi[:], AF.Ln)
            a.activation(rsi[:], rsi[:], AF.Exp, scale=-0.5)
            sqg = small.tile([B, 512], F32)
            v.tensor_mul(sqg[:], gp_sb[:], gp_sb[:])
            ssqg = small.tile([B, 2], F32)
            v.reduce_sum(ssqg[:], sqg.rearrange("p (t c) -> p t c", t=2),
                         axis=mybir.AxisListType.X)
            rsg = small.tile([B, 2], F32)
            a.activation(rsg[:], ssqg[:], AF.Ln)
            a.activation(rsg[:], rsg[:], AF.Exp, scale=-0.5)
            # normalize rows in place (anchor sums already consumed ind_sb)
            for p in range(4):
                v.tensor_scalar_mul(ind_sb[:, p * C:(p + 1) * C],
                                    ind_sb[:, p * C:(p + 1) * C], rsi[:, p:p + 1])
            for t in range(2):
                v.tensor_scalar_mul(gp_sb[:, t * C:(t + 1) * C],
                                    gp_sb[:, t * C:(t + 1) * C], rsg[:, t:t + 1])
            D = small.tile([B, 8], F32)
            prod = small.tile([B, C], F32)
            for p in range(4):
                for t in range(2):
                    col = p * 2 + t
                    v.tensor_mul(prod[:], ind_sb[:, p * C:(p + 1) * C],
                                 gp_sb[:, t * C:(t + 1) * C])
                    v.reduce_sum(D[:, col:col + 1], prod[:],
                                 axis=mybir.AxisListType.X)
            U = small.tile([B, 8], F32)
            a.activation(U[:], D[:], AF.Abs)
            # mask==1 entries (cols 0,1,4,7) use -sim instead of |sim|
            v.tensor_scalar_mul(U[:, 0:2], D[:, 0:2], -1.0)
            v.tensor_scalar_mul(U[:, 4:5], D[:, 4:5], -1.0)
            v.tensor_scalar_mul(U[:, 7:8], D[:, 7:8], -1.0)
            isum = small.tile([B, 1], F32)
            v.reduce_sum(isum[:], U[:], axis=mybir.AxisListType.X)
            # contrib_b = -negl/B + isum/(8B) + 1/B  -> sums to nll + intra_loss
            c1 = small.tile([B, 1], F32)
            c2 = small.tile([B, 1], F32)
            contrib = small.tile([B, 1], F32)
            v.tensor_scalar_mul(c1[:], negl[:], -1.0 / B)
            v.tensor_scalar_mul(c2[:], isum[:], 1.0 / (8 * B))
            v.tensor_add(contrib[:], c1[:], c2[:])
            v.tensor_scalar_add(contrib[:], contrib[:], 1.0 / B)

            # ---------- main loop over bank tiles ----------
            E_sb = small.tile([B, K], F32)
            v.memset(E_sb[:], 0.0)
            for t in range(TILES):
                k = t // TILES_PER_CLASS
                T_sb = tpool.tile([128, 4096], BF16)
                src = bank[t * NT:(t + 1) * NT, :].rearrange("(p e) x -> p e x", e=EPG)
                # SWDGE cast-DMA: f32 HBM -> bf16 SBUF at line rate
                nc.gpsimd.dma_start(out=T_sb.rearrange("p (e x) -> p e x", e=EPG),
                                    in_=src)
                Tv = T_sb.rearrange("p (e j c) -> p e j c", e=EPG, j=4)
                S_sb = spool.tile([128, 1024], BF16)
                Sv = S_sb.rearrange("p (e c) -> p e c", e=EPG)
                tmp = spool.tile([128, 1024], BF16, tag="tmp")
                tv = tmp.rearrange("p (e c) -> p e c", e=EPG)
                v.tensor_add(Sv[:], Tv[:, :, 0, :], Tv[:, :, 1, :])
                v.tensor_add(tv[:], Tv[:, :, 2, :], Tv[:, :, 3, :])
                v.tensor_add(Sv[:], Sv[:], tv[:])
                ssum4 = spool.tile([128, EPG], F32, tag="ssum")
                sqscr = sqpool.tile([128, C], F32)
                for e in range(EPG):
                    a.activation(sqscr[:], Sv[:, e, :], AF.Square,
                                 accum_out=ssum4[:, e:e + 1])
                rh4 = spool.tile([128, EPG], F32, tag="rh4")
                a.activation(rh4[:], ssum4[:], AF.Ln)
                a.activation(rh4[:], rh4[:], AF.Exp, scale=-0.5)
                # normalize S rows in place (per-entry 1/||S||)
                for e in range(EPG):
                    v.tensor_scalar_mul(Sv[:, e, :], Sv[:, e, :], rh4[:, e:e + 1])
                # transpose all 4 e-groups into [c, n=512] chunks (h = c-half)
                st_ps = [ps_st.tile([128, 512], BF16, name="stps", tag="stps")
                         for _ in range(2)]
                for e in range(EPG):
                    for h in range(2):
                        nc.tensor.transpose(
                            st_ps[h][:, e * 128:(e + 1) * 128],
                            S_sb[:, e * C + h * 128: e * C + (h + 1) * 128],
                            ident_bf[:])
                p_ps = ps_p.tile([B, 512], F32)
                for h in range(2):
                    st_sb = stpool.tile([128, 512], BF16)
                    a.copy(st_sb[:], st_ps[h][:])
                    nc.tensor.matmul(p_ps[:], at_sb[:, h, :], st_sb[:],
                                     start=(h == 0), stop=(h == 1))
                e_sb = epool.tile([B, 512], F32)
                a.activation(e_sb[:], p_ps[:], AF.Exp, scale=0.5)
                et = epool.tile([B, 1], F32, tag="et")
                v.reduce_sum(et[:], e_sb[:], axis=mybir.AxisListType.X)
                v.tensor_add(E_sb[:, k:k + 1], E_sb[:, k:k + 1], et[:])

            # ---------- epilogue: partial scalars ----------
            nc.sync.dma_start(out=e_dbg[:], in_=E_sb[:])
            t4b = small.tile([B, K], F32)
            epb = small.tile([B, 1], F32)
            rsum = small.tile([B, 1], F32)
            enb = small.tile([B, 1], F32)
            v.tensor_mul(t4b[:], E_sb[:], oh4_sb[:])
            v.reduce_sum(epb[:], t4b[:], axis=mybir.AxisListType.X)
            v.reduce_sum(rsum[:], E_sb[:], axis=mybir.AxisListType.X)
            v.tensor_scalar_mul(enb[:], epb[:], -1.0)
            v.tensor_add(enb[:], enb[:], rsum[:])
            F = small.tile([B, 4], F32)
            v.memset(F[:], 0.0)
            v.tensor_scalar_mul(F[:, 0:1], epb[:], 1.0 / (B * NB))
            v.tensor_scalar_mul(F[:, 1:2], enb[:], 1.0 / (B * (K - 1) * NB))
            v.tensor_copy(F[:, 2:3], contrib[:])
            out_ps = ps_one.tile([4, 1], F32, tag="o3")
            nc.tensor.matmul(out_ps[:], F[:], ones_sb[0:B, :], start=True, stop=True)
            out_sb = small.tile([4, 1], F32)
            a.copy(out_sb[:], out_ps[:])
            nc.sync.dma_start(out=out_d[:], in_=out_sb[:])

    nc.compile()
    return nc


_NC = None


def _get_nc():
    global _NC
    if _NC is None:
        _NC = _build()
    return _NC


def _make_in_maps(hazards, S, indiv, gene, path, cohort_bank, label, c):
    hazards = np.asarray(hazards, dtype=np.float32)
    S = np.asarray(S, dtype=np.float32)
    indiv = np.asarray(indiv, dtype=np.float32)
    gene = np.asarray(gene, dtype=np.float32)
    path = np.asarray(path, dtype=np.float32)
    cohort_bank = np.asarray(cohort_bank, dtype=np.float32)
    label = np.asarray(label)
    c = np.asarray(c)

    oh5 = np.zeros((B, K + 1), np.float32)
    oh5[np.arange(B), label] = 1.0
    oh5b = np.zeros((B, K + 1), np.float32)
    oh5b[np.arange(B), label + 1] = 1.0
    oh4 = oh5[:, :K].copy()
    spad = np.concatenate([np.ones((B, 1), np.float32), S], axis=1)
    cfs = np.stack([c.astype(np.float32), 1.0 - c.astype(np.float32)], axis=1)
    common = dict(
        indiv=np.ascontiguousarray(indiv.reshape(B, -1)),
        gp=np.ascontiguousarray(
            np.concatenate([gene.reshape(B, -1), path.reshape(B, -1)], axis=1)),
        haz=np.ascontiguousarray(hazards),
        spad=np.ascontiguousarray(spad),
        ohy=oh5, ohy1=oh5b, oh4=oh4, cfs=np.ascontiguousarray(cfs),
    )
    bankf = cohort_bank.reshape(K, NB, 1024)
    in_maps = []
    for i in range(NCORES):
        shard = np.ascontiguousarray(
            bankf[:, i * NSH:(i + 1) * NSH, :]).reshape(ROWS, 1024)
        in_maps.append({**common, "bank": shard})
    return in_maps


_LAST_RESULTS = None  # stashed for test.py introspection


def kernel(hazards, S, indiv, gene, path, cohort_bank, label, c):
    global _LAST_RESULTS
    nc = _get_nc()
    in_maps = _make_in_maps(hazards, S, indiv, gene, path, cohort_bank, label, c)
    trace = bool(int(os.environ.get("TRNK_TRACE", "0")))
    res = run_bass_kernel_spmd(nc, in_maps, core_ids=list(range(NCORES)),
                               trace=trace)
    _LAST_RESULTS = res
    outs = np.stack([r["out_vec"][:, 0] for r in res.results])  # [8, 4]
    ep = float(outs[:, 0].sum())
    en = float(outs[:, 1].sum())
    other = float(outs[:, 2].mean())
    loss = other - math.log((ep + EPS_COH) / (ep + en + EPS_COH))
    return np.float32(loss)



# revision 3
# speedup vs baseline: 1.2143x; 1.2143x over previous
"""Trainium2 Bass kernel for nn_Loss_factory_12429635355015.

Loss = NLLSurv + CohortLoss(intra + inter) over a [4, 8192, 4, 256] cohort bank.

Strategy (memory-bound, 8 NeuronCores):
  - Shard cohort_bank along the N (bank-entry) axis: each core streams its
    16 MiB shard once at HBM line rate (8 tiles x 2 MiB contiguous SWDGE
    cast-DMAs, f32 HBM -> bf16 SBUF).
  - Per 512-entry tile the compute is balanced so every engine stays under
    the ~5.9us/tile DMA floor:
      DVE:  2 adds for the 4-component sum; segmented reduce for ||S||^2;
            4 diag builds; 2 PSUM->SBUF copies of the transposed tile.
      ACT:  one big Square; Ln/Exp rsqrt; Exp(sims/tau) with accum_out
            (the per-tile exp-sum comes from the ACT accumulator, no
            separate reduction).
      PE :  transpose fused with the l2-norm scale (matmul against
            diag(rsqrt(ssq)) instead of identity), then the anchor matmul.
  - NLL + intra terms are computed on-device from host-encoded one-hots
    (index encoding only; all arithmetic on device).
  - Each core outputs [ep_partial, en_partial, nll+intra]; the host sums the
    two scalars across cores (the 'all-reduce two scalars' step) and applies
    the final -log((ep+eps)/(ep+en+eps)).
"""

import math
import os
import sys

import numpy as np

for _p in ("/opt/trn_rl_repo",):
    if _p not in sys.path and os.path.isdir(_p):
        sys.path.insert(0, _p)

import concourse.bacc as bacc
import concourse.tile as tile
from concourse import mybir
from concourse.bass_utils import run_bass_kernel_spmd

# Pin every activation to the one table set that contains all functions this
# kernel uses (Square/Ln/Exp/Copy/Abs/Identity). Without this, Bacc's
# first-match set selection alternates between sets (Ln lives outside the
# default exp set) and reloads the ACT tables ~1.3us per switch every tile.
_ACT_SET = "natural_log_exp_and_others"


def _pin_act_tables():
    import functools
    import concourse.hw_specs as hw_specs
    if getattr(hw_specs.get_activation_tables, "_pinned", False):
        return
    orig = hw_specs.get_activation_tables

    @functools.cache
    def pinned(arch):
        tabs = orig(arch)
        return {k: (v if k == _ACT_SET else set()) for k, v in tabs.items()}

    pinned._pinned = True
    hw_specs.get_activation_tables = pinned
    bacc.get_activation_tables = pinned


_pin_act_tables()

F32 = mybir.dt.float32
AF = mybir.ActivationFunctionType

# Problem constants (hardcoded per spec).
B = 64            # batch
K = 4             # n_cls
C = 256           # feature dim
NB = 8192         # bank entries per class (global)
NCORES = 8
NSH = NB // NCORES          # 1024 bank entries per class per core
ROWS = K * NSH              # 4096 rows of [4*256] per core
NT = 512                    # bank entries per tile (2 MiB)
TILES = ROWS // NT          # 8
TILES_PER_CLASS = NSH // NT # 2
EPG = NT // 128             # 4 entries per partition per tile
EPS_NLL = 1e-7
EPS_COH = 1e-8


def _build():
    nc = bacc.Bacc("TRN2", target_bir_lowering=False, debug=False,
                   enable_asserts=False, num_devices=NCORES)

    bank = nc.dram_tensor("bank", [ROWS, 1024], F32, kind="ExternalInput")
    indiv = nc.dram_tensor("indiv", [B, 1024], F32, kind="ExternalInput")
    gp = nc.dram_tensor("gp", [B, 512], F32, kind="ExternalInput")
    haz = nc.dram_tensor("haz", [B, K], F32, kind="ExternalInput")
    spad = nc.dram_tensor("spad", [B, K + 1], F32, kind="ExternalInput")
    ohy = nc.dram_tensor("ohy", [B, K + 1], F32, kind="ExternalInput")
    ohy1 = nc.dram_tensor("ohy1", [B, K + 1], F32, kind="ExternalInput")
    oh4 = nc.dram_tensor("oh4", [B, K], F32, kind="ExternalInput")
    cfs = nc.dram_tensor("cfs", [B, 2], F32, kind="ExternalInput")

    out_d = nc.dram_tensor("out_vec", [4, 1], F32, kind="ExternalOutput")
    e_dbg = nc.dram_tensor("e_dbg", [B, K], F32, kind="ExternalOutput")

    import ml_dtypes
    ident_d = nc.inline_tensor(np.eye(128, dtype=np.float32), "ident")
    ident_bf_d = nc.inline_tensor(np.eye(128, dtype=ml_dtypes.bfloat16), "ident_bf")
    ones_d = nc.inline_tensor(np.ones((128, 1), dtype=np.float32), "ones_col")

    v = nc.vector
    a = nc.scalar

    with tile.TileContext(nc) as tc:
        from contextlib import ExitStack
        with ExitStack() as ctx:
            const = ctx.enter_context(tc.tile_pool(name="const", bufs=1))
            small = ctx.enter_context(tc.tile_pool(name="small", bufs=1))
            tpool = ctx.enter_context(tc.tile_pool(name="T", bufs=3))
            upool = ctx.enter_context(tc.tile_pool(name="U", bufs=2))
            spool = ctx.enter_context(tc.tile_pool(name="S", bufs=2))
            sqpool = ctx.enter_context(tc.tile_pool(name="sq", bufs=2))
            dpool = ctx.enter_context(tc.tile_pool(name="dg", bufs=2))
            stpool = ctx.enter_context(tc.tile_pool(name="STsb", bufs=2))
            epool = ctx.enter_context(tc.tile_pool(name="esb", bufs=2))
            ps_st = ctx.enter_context(tc.tile_pool(name="ps_st", bufs=2, space="PSUM"))
            ps_p = ctx.enter_context(tc.tile_pool(name="ps_p", bufs=2, space="PSUM"))
            ps_one = ctx.enter_context(tc.tile_pool(name="ps_one", bufs=1, space="PSUM"))

            BF16 = mybir.dt.bfloat16
            ident_sb = const.tile([128, 128], F32)
            nc.sync.dma_start(out=ident_sb[:], in_=ident_d[:])
            ident_bf = const.tile([128, 128], BF16)
            nc.sync.dma_start(out=ident_bf[:], in_=ident_bf_d[:])
            ones_sb = const.tile([128, 1], F32)
            nc.sync.dma_start(out=ones_sb[:], in_=ones_d[:])

            # ---------- anchors: A = l2norm(mean_j indiv[b,j,:]) ----------
            ind_sb = small.tile([B, 1024], F32)
            nc.sync.dma_start(out=ind_sb[:], in_=indiv[:])
            iv = ind_sb.rearrange("p (j c) -> p j c", j=4)
            asum = small.tile([B, C], F32)
            atmp = small.tile([B, C], F32)
            v.tensor_add(asum[:], iv[:, 0, :], iv[:, 1, :])
            v.tensor_add(atmp[:], iv[:, 2, :], iv[:, 3, :])
            v.tensor_add(asum[:], asum[:], atmp[:])
            sqa = small.tile([B, C], F32)
            ssa = small.tile([B, 1], F32)
            a.activation(sqa[:], asum[:], AF.Square, accum_out=ssa[:])
            lna = small.tile([B, 1], F32)
            a.activation(lna[:], ssa[:], AF.Ln)
            rsa = small.tile([B, 1], F32)
            a.activation(rsa[:], lna[:], AF.Exp, scale=-0.5)
            v.tensor_scalar_mul(asum[:], asum[:], rsa[:])
            at_ps = ps_one.tile([128, 2, B], F32, tag="at")
            for h in range(2):
                nc.tensor.transpose(at_ps[:, h, :], asum[:, h * 128:(h + 1) * 128],
                                    ident_sb[0:B, 0:B])
            at_sb = const.tile([128, 2, B], BF16)
            a.copy(at_sb[:], at_ps[:])

            # ---------- NLL (per-b, b on partitions) ----------
            haz_sb = small.tile([B, K], F32)
            nc.sync.dma_start(out=haz_sb[:], in_=haz[:])
            spad_sb = small.tile([B, K + 1], F32)
            nc.sync.dma_start(out=spad_sb[:], in_=spad[:])
            ohy_sb = small.tile([B, K + 1], F32)
            nc.sync.dma_start(out=ohy_sb[:], in_=ohy[:])
            ohy1_sb = small.tile([B, K + 1], F32)
            nc.sync.dma_start(out=ohy1_sb[:], in_=ohy1[:])
            oh4_sb = small.tile([B, K], F32)
            nc.sync.dma_start(out=oh4_sb[:], in_=oh4[:])
            cfs_sb = small.tile([B, 2], F32)
            nc.sync.dma_start(out=cfs_sb[:], in_=cfs[:])

            t5 = small.tile([B, K + 1], F32)
            t4 = small.tile([B, K], F32)
            sy = small.tile([B, 1], F32)
            hy = small.tile([B, 1], F32)
            sy1 = small.tile([B, 1], F32)
            v.tensor_mul(t5[:], spad_sb[:], ohy_sb[:])
            v.reduce_sum(sy[:], t5[:], axis=mybir.AxisListType.X)
            v.tensor_mul(t4[:], haz_sb[:], ohy_sb[:, 0:K])
            v.reduce_sum(hy[:], t4[:], axis=mybir.AxisListType.X)
            v.tensor_mul(t5[:], spad_sb[:], ohy1_sb[:])
            v.reduce_sum(sy1[:], t5[:], axis=mybir.AxisListType.X)
            for x in (sy, hy, sy1):
                v.tensor_scalar_max(x[:], x[:], EPS_NLL)
            lsy = small.tile([B, 1], F32)
            lhy = small.tile([B, 1], F32)
            lsy1 = small.tile([B, 1], F32)
            a.activation(lsy[:], sy[:], AF.Ln)
            a.activation(lhy[:], hy[:], AF.Ln)
            a.activation(lsy1[:], sy1[:], AF.Ln)
            tu = small.tile([B, 1], F32)
            tcen = small.tile([B, 1], F32)
            negl = small.tile([B, 1], F32)
            v.tensor_add(tu[:], lsy[:], lhy[:])
            v.tensor_mul(tu[:], tu[:], cfs_sb[:, 1:2])      # *(1-cf)
            v.tensor_mul(tcen[:], lsy1[:], cfs_sb[:, 0:1])  # *cf
            v.tensor_add(negl[:], tu[:], tcen[:])           # = -neg_l per b

            # ---------- intra cohort term ----------
            gp_sb = small.tile([B, 512], F32)
            nc.sync.dma_start(out=gp_sb[:], in_=gp[:])
            sqi = small.tile([B, 1024], F32)
            v.tensor_mul(sqi[:], ind_sb[:], ind_sb[:])
            ssqi = small.tile([B, 4], F32)
            v.reduce_sum(ssqi[:], sqi.rearrange("p (j c) -> p j c", j=4),
                         axis=mybir.AxisListType.X)
            rsi = small.tile([B, 4], F32)
            a.activation(rsi[:], ssqi[:], AF.Ln)
            a.activation(rsi[:], rsi[:], AF.Exp, scale=-0.5)
            sqg = small.tile([B, 512], F32)
            v.tensor_mul(sqg[:], gp_sb[:], gp_sb[:])
            ssqg = small.tile([B, 2], F32)
            v.reduce_sum(ssqg[:], sqg.rearrange("p (t c) -> p t c", t=2),
                         axis=mybir.AxisListType.X)
            rsg = small.tile([B, 2], F32)
            a.activation(rsg[:], ssqg[:], AF.Ln)
            a.activation(rsg[:], rsg[:], AF.Exp, scale=-0.5)
            # normalize rows in place (anchor sums already consumed ind_sb)
            for p in range(4):
                v.tensor_scalar_mul(ind_sb[:, p * C:(p + 1) * C],
                                    ind_sb[:, p * C:(p + 1) * C], rsi[:, p:p + 1])
            for t in range(2):
                v.tensor_scalar_mul(gp_sb[:, t * C:(t + 1) * C],
                                    gp_sb[:, t * C:(t + 1) * C], rsg[:, t:t + 1])
            D = small.tile([B, 8], F32)
            prod = small.tile([B, C], F32)
            for p in range(4):
                for t in range(2):
                    col = p * 2 + t
                    v.tensor_mul(prod[:], ind_sb[:, p * C:(p + 1) * C],
                                 gp_sb[:, t * C:(t + 1) * C])
                    v.reduce_sum(D[:, col:col + 1], prod[:],
                                 axis=mybir.AxisListType.X)
            U8 = small.tile([B, 8], F32)
            a.activation(U8[:], D[:], AF.Abs)
            # mask==1 entries (cols 0,1,4,7) use -sim instead of |sim|
            v.tensor_scalar_mul(U8[:, 0:2], D[:, 0:2], -1.0)
            v.tensor_scalar_mul(U8[:, 4:5], D[:, 4:5], -1.0)
            v.tensor_scalar_mul(U8[:, 7:8], D[:, 7:8], -1.0)
            isum = small.tile([B, 1], F32)
            v.reduce_sum(isum[:], U8[:], axis=mybir.AxisListType.X)
            # contrib_b = -negl/B + isum/(8B) + 1/B  -> sums to nll + intra_loss
            c1 = small.tile([B, 1], F32)
            c2 = small.tile([B, 1], F32)
            contrib = small.tile([B, 1], F32)
            v.tensor_scalar_mul(c1[:], negl[:], -1.0 / B)
            v.tensor_scalar_mul(c2[:], isum[:], 1.0 / (8 * B))
            v.tensor_add(contrib[:], c1[:], c2[:])
            v.tensor_scalar_add(contrib[:], contrib[:], 1.0 / B)

            # ---------- main loop over bank tiles ----------
            E_sb = small.tile([B, K], F32)
            v.memset(E_sb[:], 0.0)
            for t in range(TILES):
                k = t // TILES_PER_CLASS
                T_sb = tpool.tile([128, 4096], BF16)
                src = bank[t * NT:(t + 1) * NT, :].rearrange("(p e) x -> p e x", e=EPG)
                # SWDGE cast-DMA: f32 HBM -> bf16 SBUF at line rate
                nc.gpsimd.dma_start(out=T_sb.rearrange("p (e x) -> p e x", e=EPG),
                                    in_=src)
                Tv = T_sb.rearrange("p (e j c) -> p e j c", e=EPG, j=4)
                # component sum over j in 2 adds: (j0+j2, j1+j3) then pair-sum
                U_sb = upool.tile([128, 2048], BF16)
                Uv = U_sb.rearrange("p (e g c) -> p e g c", e=EPG, g=2)
                v.tensor_add(Uv[:], Tv[:, :, 0:2, :], Tv[:, :, 2:4, :])
                S_sb = spool.tile([128, 1024], BF16)
                Sv = S_sb.rearrange("p (e c) -> p e c", e=EPG)
                v.tensor_add(Sv[:], Uv[:, :, 0, :], Uv[:, :, 1, :])
                # per-entry sum of squares: one big Square, one segmented reduce
                sq_sb = sqpool.tile([128, 1024], BF16)
                a.activation(sq_sb[:], S_sb[:], AF.Square)
                ssq4 = sqpool.tile([128, EPG], F32, tag="ssq")
                v.reduce_sum(ssq4[:], sq_sb.rearrange("p (e c) -> p e c", e=EPG),
                             axis=mybir.AxisListType.X)
                rh4 = sqpool.tile([128, EPG], F32, tag="rh4")
                a.activation(rh4[:], ssq4[:], AF.Ln)
                a.activation(rh4[:], rh4[:], AF.Exp, scale=-0.5)
                # scaled-transpose: rhs = diag(1/||S||) folds the l2norm into PE
                dg = dpool.tile([128, EPG, 128], BF16)
                for e in range(EPG):
                    v.tensor_scalar_mul(dg[:, e, :], ident_bf[:], rh4[:, e:e + 1])
                st_ps = [ps_st.tile([128, 512], F32, name=f"stps{h}", tag=f"stps{h}")
                         for h in range(2)]
                for e in range(EPG):
                    for h in range(2):
                        nc.tensor.matmul(
                            st_ps[h][:, e * 128:(e + 1) * 128],
                            S_sb[:, e * C + h * 128: e * C + (h + 1) * 128],
                            dg[:, e, :], start=True, stop=True)
                p_ps = ps_p.tile([B, 512], F32)
                for h in range(2):
                    st_sb = stpool.tile([128, 512], BF16, name=f"st{h}", tag=f"st{h}")
                    v.tensor_copy(st_sb[:], st_ps[h][:])
                    nc.tensor.matmul(p_ps[:], at_sb[:, h, :], st_sb[:],
                                     start=(h == 0), stop=(h == 1))
                e_scr = epool.tile([B, 512], BF16)
                et = epool.tile([B, 1], F32, tag="et")
                a.activation(e_scr[:], p_ps[:], AF.Exp, scale=0.5, accum_out=et[:])
                v.tensor_add(E_sb[:, k:k + 1], E_sb[:, k:k + 1], et[:])

            # ---------- epilogue: partial scalars ----------
            nc.sync.dma_start(out=e_dbg[:], in_=E_sb[:])
            t4b = small.tile([B, K], F32)
            epb = small.tile([B, 1], F32)
            rsum = small.tile([B, 1], F32)
            enb = small.tile([B, 1], F32)
            v.tensor_mul(t4b[:], E_sb[:], oh4_sb[:])
            v.reduce_sum(epb[:], t4b[:], axis=mybir.AxisListType.X)
            v.reduce_sum(rsum[:], E_sb[:], axis=mybir.AxisListType.X)
            v.tensor_scalar_mul(enb[:], epb[:], -1.0)
            v.tensor_add(enb[:], enb[:], rsum[:])
            F = small.tile([B, 4], F32)
            v.memset(F[:], 0.0)
            v.tensor_scalar_mul(F[:, 0:1], epb[:], 1.0 / (B * NB))
            v.tensor_scalar_mul(F[:, 1:2], enb[:], 1.0 / (B * (K - 1) * NB))
            v.tensor_copy(F[:, 2:3], contrib[:])
            out_ps = ps_one.tile([4, 1], F32, tag="o3")
            nc.tensor.matmul(out_ps[:], F[:], ones_sb[0:B, :], start=True, stop=True)
            out_sb = small.tile([4, 1], F32)
            a.copy(out_sb[:], out_ps[:])
            nc.sync.dma_start(out=out_d[:], in_=out_sb[:])

    nc.compile()
    return nc


_NC = None


def _get_nc():
    global _NC
    if _NC is None:
        _NC = _build()
    return _NC


def _make_in_maps(hazards, S, indiv, gene, path, cohort_bank, label, c):
    hazards = np.asarray(hazards, dtype=np.float32)
    S = np.asarray(S, dtype=np.float32)
    indiv = np.asarray(indiv, dtype=np.float32)
    gene = np.asarray(gene, dtype=np.float32)
    path = np.asarray(path, dtype=np.float32)
    cohort_bank = np.asarray(cohort_bank, dtype=np.float32)
    label = np.asarray(label)
    c = np.asarray(c)

    oh5 = np.zeros((B, K + 1), np.float32)
    oh5[np.arange(B), label] = 1.0
    oh5b = np.zeros((B, K + 1), np.float32)
    oh5b[np.arange(B), label + 1] = 1.0
    oh4 = oh5[:, :K].copy()
    spad = np.concatenate([np.ones((B, 1), np.float32), S], axis=1)
    cfs = np.stack([c.astype(np.float32), 1.0 - c.astype(np.float32)], axis=1)
    common = dict(
        indiv=np.ascontiguousarray(indiv.reshape(B, -1)),
        gp=np.ascontiguousarray(
            np.concatenate([gene.reshape(B, -1), path.reshape(B, -1)], axis=1)),
        haz=np.ascontiguousarray(hazards),
        spad=np.ascontiguousarray(spad),
        ohy=oh5, ohy1=oh5b, oh4=oh4, cfs=np.ascontiguousarray(cfs),
    )
    bankf = cohort_bank.reshape(K, NB, 1024)
    in_maps = []
    for i in range(NCORES):
        shard = np.ascontiguousarray(
            bankf[:, i * NSH:(i + 1) * NSH, :]).reshape(ROWS, 1024)
        in_maps.append({**common, "bank": shard})
    return in_maps


_LAST_RESULTS = None  # stashed for test.py introspection


def kernel(hazards, S, indiv, gene, path, cohort_bank, label, c):
    global _LAST_RESULTS
    nc = _get_nc()
    in_maps = _make_in_maps(hazards, S, indiv, gene, path, cohort_bank, label, c)
    trace = bool(int(os.environ.get("TRNK_TRACE", "0")))
    res = run_bass_kernel_spmd(nc, in_maps, core_ids=list(range(NCORES)),
                               trace=trace)
    _LAST_RESULTS = res
    outs = np.stack([r["out_vec"][:, 0] for r in res.results])  # [8, 4]
    ep = float(outs[:, 0].sum())
    en = float(outs[:, 1].sum())
    other = float(outs[:, 2].mean())
    loss = other - math.log((ep + EPS_COH) / (ep + en + EPS_COH))
    return np.float32(loss)


# revision 14
# speedup vs baseline: 1.4400x; 1.1858x over previous
"""Trainium2 Bass kernel for nn_Loss_factory_12429635355015.

Loss = NLLSurv + CohortLoss(intra + inter) over a [4, 8192, 4, 256] cohort bank.

Strategy (memory-bound, 8 NeuronCores):
  - Shard cohort_bank along the N (bank-entry) axis: each core streams its
    16 MiB shard once at HBM line rate (8 tiles x 2 MiB contiguous SWDGE
    cast-DMAs, f32 HBM -> bf16 SBUF).
  - Per 512-entry tile the compute is balanced so every engine stays under
    the ~5.9us/tile DMA floor:
      DVE:  2 adds for the 4-component sum + segmented reduce for ||S||^2.
      PE :  8 transposes, then the anchor matmul in TRANSPOSED orientation:
            P_T[n, b] = st.T @ at, so the out-partition of block e is the
            original bank partition p.
      ACT:  Square; rh4 = exp(-0.5 ln ssq + ln .5) = 0.5/||S||; then
            Exp(P_T, scale=rh4[:, e]) fuses the l2-norm AND 1/tau into the
            exp's per-partition scale. Per-class sums are PE ones-matmuls
            accumulated in PSUM across each class's tiles.
  - NLL + intra terms are computed on-device from host-encoded one-hots
    (index encoding only; all arithmetic on device).
  - Each core outputs [ep_partial, en_partial, nll+intra]; the host sums the
    two scalars across cores (the 'all-reduce two scalars' step) and applies
    the final -log((ep+eps)/(ep+en+eps)).
"""

import math
import os
import sys

import numpy as np

for _p in ("/opt/trn_rl_repo",):
    if _p not in sys.path and os.path.isdir(_p):
        sys.path.insert(0, _p)

import concourse.bacc as bacc
import concourse.tile as tile
from concourse import mybir
from concourse.bass_utils import run_bass_kernel_spmd

# Pin every activation to the one table set that contains all functions this
# kernel uses (Square/Ln/Exp/Copy/Abs/Identity). Without this, Bacc's
# first-match set selection alternates between sets (Ln lives outside the
# default exp set) and reloads the ACT tables ~1.3us per switch every tile.
_ACT_SET = "natural_log_exp_and_others"


def _pin_act_tables():
    import functools
    import concourse.hw_specs as hw_specs
    if getattr(hw_specs.get_activation_tables, "_pinned", False):
        return
    orig = hw_specs.get_activation_tables

    @functools.cache
    def pinned(arch):
        tabs = orig(arch)
        return {k: (v if k == _ACT_SET else set()) for k, v in tabs.items()}

    pinned._pinned = True
    hw_specs.get_activation_tables = pinned
    bacc.get_activation_tables = pinned


_pin_act_tables()

F32 = mybir.dt.float32
AF = mybir.ActivationFunctionType

# Problem constants (hardcoded per spec).
B = 64            # batch
K = 4             # n_cls
C = 256           # feature dim
NB = 8192         # bank entries per class (global)
NCORES = 8
NSH = NB // NCORES          # 1024 bank entries per class per core
ROWS = K * NSH              # 4096 rows of [4*256] per core
NT = 512                    # bank entries per tile (2 MiB)
TILES = ROWS // NT          # 8
TILES_PER_CLASS = NSH // NT # 2
EPG = NT // 128             # 4 entries per partition per tile
LN_HALF = math.log(0.5)
EPS_NLL = 1e-7
EPS_COH = 1e-8


def _build():
    nc = bacc.Bacc("TRN2", target_bir_lowering=False, debug=False,
                   enable_asserts=False, num_devices=NCORES)

    bank = nc.dram_tensor("bank", [ROWS, 1024], F32, kind="ExternalInput")
    indiv = nc.dram_tensor("indiv", [B, 1024], F32, kind="ExternalInput")
    gp = nc.dram_tensor("gp", [B, 512], F32, kind="ExternalInput")
    haz = nc.dram_tensor("haz", [B, K], F32, kind="ExternalInput")
    spad = nc.dram_tensor("spad", [B, K + 1], F32, kind="ExternalInput")
    ohy = nc.dram_tensor("ohy", [B, K + 1], F32, kind="ExternalInput")
    ohy1 = nc.dram_tensor("ohy1", [B, K + 1], F32, kind="ExternalInput")
    oh4t = nc.dram_tensor("oh4t", [1, K * B], F32, kind="ExternalInput")
    cfs = nc.dram_tensor("cfs", [B, 2], F32, kind="ExternalInput")

    out_d = nc.dram_tensor("out_vec", [1, 4], F32, kind="ExternalOutput")
    e_dbg = nc.dram_tensor("e_dbg", [1, K * B], F32, kind="ExternalOutput")

    import ml_dtypes
    ident_d = nc.inline_tensor(np.eye(128, dtype=np.float32), "ident")
    ident_bf_d = nc.inline_tensor(np.eye(128, dtype=ml_dtypes.bfloat16), "ident_bf")
    ones_d = nc.inline_tensor(np.ones((128, 1), dtype=np.float32), "ones_col")
    ones_bf_d = nc.inline_tensor(np.ones((128, 1), dtype=ml_dtypes.bfloat16),
                                 "ones_col_bf")

    v = nc.vector
    a = nc.scalar

    with tile.TileContext(nc) as tc:
        from contextlib import ExitStack
        with ExitStack() as ctx:
            const = ctx.enter_context(tc.tile_pool(name="const", bufs=1))
            small = ctx.enter_context(tc.tile_pool(name="small", bufs=1))
            tpool = ctx.enter_context(tc.tile_pool(name="T", bufs=3))
            upool = ctx.enter_context(tc.tile_pool(name="U", bufs=2))
            spool = ctx.enter_context(tc.tile_pool(name="S", bufs=2))
            sqpool = ctx.enter_context(tc.tile_pool(name="sq", bufs=2))
            stpool = ctx.enter_context(tc.tile_pool(name="STsb", bufs=2))
            epool = ctx.enter_context(tc.tile_pool(name="esb", bufs=2))
            ps_st = ctx.enter_context(tc.tile_pool(name="ps_st", bufs=1, space="PSUM"))
            ps_p = ctx.enter_context(tc.tile_pool(name="ps_p", bufs=2, space="PSUM"))
            ps_one = ctx.enter_context(tc.tile_pool(name="ps_one", bufs=1, space="PSUM"))

            BF16 = mybir.dt.bfloat16
            ident_sb = const.tile([128, 128], F32)
            nc.sync.dma_start(out=ident_sb[:], in_=ident_d[:])
            ident_bf = const.tile([128, 128], BF16)
            nc.sync.dma_start(out=ident_bf[:], in_=ident_bf_d[:])
            ones_sb = const.tile([128, 1], F32)
            nc.sync.dma_start(out=ones_sb[:], in_=ones_d[:])
            ones_bf = const.tile([128, 1], BF16)
            nc.sync.dma_start(out=ones_bf[:], in_=ones_bf_d[:])
            lnhalf_sb = const.tile([128, 1], F32)
            v.memset(lnhalf_sb[:], LN_HALF)

            # ---------- anchors: A = l2norm(mean_j indiv[b,j,:]) ----------
            ind_sb = small.tile([B, 1024], F32)
            nc.sync.dma_start(out=ind_sb[:], in_=indiv[:])
            iv = ind_sb.rearrange("p (j c) -> p j c", j=4)
            asum = small.tile([B, C], F32)
            atmp = small.tile([B, C], F32)
            v.tensor_add(asum[:], iv[:, 0, :], iv[:, 1, :])
            v.tensor_add(atmp[:], iv[:, 2, :], iv[:, 3, :])
            v.tensor_add(asum[:], asum[:], atmp[:])
            sqa = small.tile([B, C], F32)
            ssa = small.tile([B, 1], F32)
            a.activation(sqa[:], asum[:], AF.Square, accum_out=ssa[:])
            lna = small.tile([B, 1], F32)
            a.activation(lna[:], ssa[:], AF.Ln)
            rsa = small.tile([B, 1], F32)
            a.activation(rsa[:], lna[:], AF.Exp, scale=-0.5)
            v.tensor_scalar_mul(asum[:], asum[:], rsa[:])
            at_ps = ps_one.tile([128, 2, B], F32, tag="at")
            for h in range(2):
                nc.tensor.transpose(at_ps[:, h, :], asum[:, h * 128:(h + 1) * 128],
                                    ident_sb[0:B, 0:B])
            at_sb = const.tile([128, 2, B], BF16)
            a.copy(at_sb[:], at_ps[:])

            # ---------- NLL (per-b, b on partitions) ----------
            haz_sb = small.tile([B, K], F32)
            nc.sync.dma_start(out=haz_sb[:], in_=haz[:])
            spad_sb = small.tile([B, K + 1], F32)
            nc.sync.dma_start(out=spad_sb[:], in_=spad[:])
            ohy_sb = small.tile([B, K + 1], F32)
            nc.sync.dma_start(out=ohy_sb[:], in_=ohy[:])
            ohy1_sb = small.tile([B, K + 1], F32)
            nc.sync.dma_start(out=ohy1_sb[:], in_=ohy1[:])
            oh4t_sb = small.tile([1, K * B], F32)
            nc.sync.dma_start(out=oh4t_sb[:], in_=oh4t[:])
            cfs_sb = small.tile([B, 2], F32)
            nc.sync.dma_start(out=cfs_sb[:], in_=cfs[:])

            t5 = small.tile([B, K + 1], F32)
            t4 = small.tile([B, K], F32)
            sy = small.tile([B, 1], F32)
            hy = small.tile([B, 1], F32)
            sy1 = small.tile([B, 1], F32)
            v.tensor_mul(t5[:], spad_sb[:], ohy_sb[:])
            v.reduce_sum(sy[:], t5[:], axis=mybir.AxisListType.X)
            v.tensor_mul(t4[:], haz_sb[:], ohy_sb[:, 0:K])
            v.reduce_sum(hy[:], t4[:], axis=mybir.AxisListType.X)
            v.tensor_mul(t5[:], spad_sb[:], ohy1_sb[:])
            v.reduce_sum(sy1[:], t5[:], axis=mybir.AxisListType.X)
            for x in (sy, hy, sy1):
                v.tensor_scalar_max(x[:], x[:], EPS_NLL)
            lsy = small.tile([B, 1], F32)
            lhy = small.tile([B, 1], F32)
            lsy1 = small.tile([B, 1], F32)
            a.activation(lsy[:], sy[:], AF.Ln)
            a.activation(lhy[:], hy[:], AF.Ln)
            a.activation(lsy1[:], sy1[:], AF.Ln)
            tu = small.tile([B, 1], F32)
            tcen = small.tile([B, 1], F32)
            negl = small.tile([B, 1], F32)
            v.tensor_add(tu[:], lsy[:], lhy[:])
            v.tensor_mul(tu[:], tu[:], cfs_sb[:, 1:2])      # *(1-cf)
            v.tensor_mul(tcen[:], lsy1[:], cfs_sb[:, 0:1])  # *cf
            v.tensor_add(negl[:], tu[:], tcen[:])           # = -neg_l per b

            # ---------- intra cohort term ----------
            gp_sb = small.tile([B, 512], F32)
            nc.sync.dma_start(out=gp_sb[:], in_=gp[:])
            sq_scr = small.tile([B, C], F32)
            ssqi = small.tile([B, 4], F32)
            for p in range(4):
                a.activation(sq_scr[:], ind_sb[:, p * C:(p + 1) * C], AF.Square,
                             accum_out=ssqi[:, p:p + 1])
            rsi = small.tile([B, 4], F32)
            a.activation(rsi[:], ssqi[:], AF.Ln)
            a.activation(rsi[:], rsi[:], AF.Exp, scale=-0.5)
            ssqg = small.tile([B, 2], F32)
            for t in range(2):
                a.activation(sq_scr[:], gp_sb[:, t * C:(t + 1) * C], AF.Square,
                             accum_out=ssqg[:, t:t + 1])
            rsg = small.tile([B, 2], F32)
            a.activation(rsg[:], ssqg[:], AF.Ln)
            a.activation(rsg[:], rsg[:], AF.Exp, scale=-0.5)
            # normalize rows in place (anchor sums already consumed ind_sb)
            for p in range(4):
                v.tensor_scalar_mul(ind_sb[:, p * C:(p + 1) * C],
                                    ind_sb[:, p * C:(p + 1) * C], rsi[:, p:p + 1])
            for t in range(2):
                v.tensor_scalar_mul(gp_sb[:, t * C:(t + 1) * C],
                                    gp_sb[:, t * C:(t + 1) * C], rsg[:, t:t + 1])
            D = small.tile([B, 8], F32)
            prod = small.tile([B, C], F32)
            for p in range(4):
                for t in range(2):
                    col = p * 2 + t
                    v.tensor_mul(prod[:], ind_sb[:, p * C:(p + 1) * C],
                                 gp_sb[:, t * C:(t + 1) * C])
                    v.reduce_sum(D[:, col:col + 1], prod[:],
                                 axis=mybir.AxisListType.X)
            U8 = small.tile([B, 8], F32)
            a.activation(U8[:], D[:], AF.Abs)
            # mask==1 entries (cols 0,1,4,7) use -sim instead of |sim|
            v.tensor_scalar_mul(U8[:, 0:2], D[:, 0:2], -1.0)
            v.tensor_scalar_mul(U8[:, 4:5], D[:, 4:5], -1.0)
            v.tensor_scalar_mul(U8[:, 7:8], D[:, 7:8], -1.0)
            isum = small.tile([B, 1], F32)
            v.reduce_sum(isum[:], U8[:], axis=mybir.AxisListType.X)
            # contrib_b = -negl/B + isum/(8B) + 1/B  -> sums to nll + intra_loss
            c1 = small.tile([B, 1], F32)
            c2 = small.tile([B, 1], F32)
            contrib = small.tile([B, 1], F32)
            v.tensor_scalar_mul(c1[:], negl[:], -1.0 / B)
            v.tensor_scalar_mul(c2[:], isum[:], 1.0 / (8 * B))
            v.tensor_add(contrib[:], c1[:], c2[:])
            v.tensor_scalar_add(contrib[:], contrib[:], 1.0 / B)

            # ---------- main loop over bank tiles ----------
            # E_ps[0, k*B + b] accumulates sum_n exp(sims[b, n in class k])
            # across each class's tiles directly in PSUM.
            E_ps = ps_one.tile([1, K, B], F32, tag="E")
            for t in range(TILES):
                k = t // TILES_PER_CLASS
                first = (t % TILES_PER_CLASS == 0)
                last = (t % TILES_PER_CLASS == TILES_PER_CLASS - 1)
                T_sb = tpool.tile([128, 4096], BF16)
                src = bank[t * NT:(t + 1) * NT, :].rearrange("(p e) x -> p e x", e=EPG)
                # SWDGE cast-DMA: f32 HBM -> bf16 SBUF at line rate
                nc.gpsimd.dma_start(out=T_sb.rearrange("p (e x) -> p e x", e=EPG),
                                    in_=src)
                Tv = T_sb.rearrange("p (e j c) -> p e j c", e=EPG, j=4)
                # component sum over j in 2 adds: (j0+j2, j1+j3) then pair-sum
                U_sb = upool.tile([128, 2048], BF16)
                Uv = U_sb.rearrange("p (e g c) -> p e g c", e=EPG, g=2)
                v.tensor_add(Uv[:], Tv[:, :, 0:2, :], Tv[:, :, 2:4, :])
                S_sb = spool.tile([128, 1024], BF16)
                Sv = S_sb.rearrange("p (e c) -> p e c", e=EPG)
                v.tensor_add(Sv[:], Uv[:, :, 0, :], Uv[:, :, 1, :])
                # per-entry sum of squares: one big Square, one segmented reduce
                sq_sb = sqpool.tile([128, 1024], BF16)
                a.activation(sq_sb[:], S_sb[:], AF.Square)
                ssq4 = sqpool.tile([128, EPG], F32, tag="ssq")
                v.reduce_sum(ssq4[:], sq_sb.rearrange("p (e c) -> p e c", e=EPG),
                             axis=mybir.AxisListType.X)
                # rh4 = 0.5/||S||  (the ln(1/2) bias folds in the 1/tau factor)
                rh4 = sqpool.tile([128, EPG], F32, tag="rh4")
                a.activation(rh4[:], ssq4[:], AF.Ln)
                a.activation(rh4[:], rh4[:], AF.Exp, scale=-0.5, bias=lnhalf_sb[:])
                st_ps = [ps_st.tile([128, 512], BF16, name=f"stps{h}", tag=f"stps{h}")
                         for h in range(2)]
                for e in range(EPG):
                    for h in range(2):
                        nc.tensor.transpose(
                            st_ps[h][:, e * 128:(e + 1) * 128],
                            S_sb[:, e * C + h * 128: e * C + (h + 1) * 128],
                            ident_bf[:])
                st_sb = [stpool.tile([128, 512], BF16, name=f"st{h}", tag=f"st{h}")
                         for h in range(2)]
                for h in range(2):
                    a.copy(st_sb[h][:], st_ps[h][:])
                # P_T[n, b] = S_n . A_b with n on partitions: out-partition of
                # block e is the original bank partition p, so rh4[:, e] is a
                # legal per-partition scale for the exp.
                p_ps = ps_p.tile([128, EPG, B], F32)
                for e in range(EPG):
                    for h in range(2):
                        nc.tensor.matmul(p_ps[:, e, :],
                                         st_sb[h][:, e * 128:(e + 1) * 128],
                                         at_sb[:, h, :],
                                         start=(h == 0), stop=(h == 1))
                e_T = epool.tile([128, EPG, B], BF16)
                for e in range(EPG):
                    a.activation(e_T[:, e, :], p_ps[:, e, :], AF.Exp,
                                 scale=rh4[:, e:e + 1])
                for e in range(EPG):
                    nc.tensor.matmul(E_ps[:, k, :], ones_bf[:], e_T[:, e, :],
                                     start=(first and e == 0),
                                     stop=(last and e == EPG - 1),
                                     skip_group_check=True)

            # ---------- epilogue: partial scalars ----------
            E_row = small.tile([1, K * B], F32)
            a.copy(E_row[:], E_ps.rearrange("p k b -> p (k b)"))
            nc.sync.dma_start(out=e_dbg[:], in_=E_row[:])
            epm = small.tile([1, K * B], F32)
            ep1 = small.tile([1, 1], F32)
            rsum = small.tile([1, 1], F32)
            en1 = small.tile([1, 1], F32)
            v.tensor_mul(epm[:], E_row[:], oh4t_sb[:])
            v.reduce_sum(ep1[:], epm[:], axis=mybir.AxisListType.X)
            v.reduce_sum(rsum[:], E_row[:], axis=mybir.AxisListType.X)
            v.tensor_scalar_mul(en1[:], ep1[:], -1.0)
            v.tensor_add(en1[:], en1[:], rsum[:])
            # contrib sum over b via PE ones-reduction
            cs_ps = ps_one.tile([1, 1], F32, tag="cs")
            nc.tensor.matmul(cs_ps[:], contrib[:], ones_sb[0:B, :],
                             start=True, stop=True)
            F1 = small.tile([1, 4], F32)
            v.memset(F1[:], 0.0)
            v.tensor_scalar_mul(F1[:, 0:1], ep1[:], 1.0 / (B * NB))
            v.tensor_scalar_mul(F1[:, 1:2], en1[:], 1.0 / (B * (K - 1) * NB))
            v.tensor_copy(F1[:, 2:3], cs_ps[:])
            nc.sync.dma_start(out=out_d[:], in_=F1[:])

    nc.compile()
    return nc


_NC = None


def _get_nc():
    global _NC
    if _NC is None:
        _NC = _build()
    return _NC


def _make_in_maps(hazards, S, indiv, gene, path, cohort_bank, label, c):
    hazards = np.asarray(hazards, dtype=np.float32)
    S = np.asarray(S, dtype=np.float32)
    indiv = np.asarray(indiv, dtype=np.float32)
    gene = np.asarray(gene, dtype=np.float32)
    path = np.asarray(path, dtype=np.float32)
    cohort_bank = np.asarray(cohort_bank, dtype=np.float32)
    label = np.asarray(label)
    c = np.asarray(c)

    oh5 = np.zeros((B, K + 1), np.float32)
    oh5[np.arange(B), label] = 1.0
    oh5b = np.zeros((B, K + 1), np.float32)
    oh5b[np.arange(B), label + 1] = 1.0
    oh4t = np.zeros((1, K * B), np.float32)
    oh4t[0, label.astype(np.int64) * B + np.arange(B)] = 1.0
    spad = np.concatenate([np.ones((B, 1), np.float32), S], axis=1)
    cfs = np.stack([c.astype(np.float32), 1.0 - c.astype(np.float32)], axis=1)
    common = dict(
        indiv=np.ascontiguousarray(indiv.reshape(B, -1)),
        gp=np.ascontiguousarray(
            np.concatenate([gene.reshape(B, -1), path.reshape(B, -1)], axis=1)),
        haz=np.ascontiguousarray(hazards),
        spad=np.ascontiguousarray(spad),
        ohy=oh5, ohy1=oh5b, oh4t=oh4t, cfs=np.ascontiguousarray(cfs),
    )
    bankf = cohort_bank.reshape(K, NB, 1024)
    in_maps = []
    for i in range(NCORES):
        shard = np.ascontiguousarray(
            bankf[:, i * NSH:(i + 1) * NSH, :]).reshape(ROWS, 1024)
        in_maps.append({**common, "bank": shard})
    return in_maps


_LAST_RESULTS = None  # stashed for test.py introspection


def kernel(hazards, S, indiv, gene, path, cohort_bank, label, c):
    global _LAST_RESULTS
    nc = _get_nc()
    in_maps = _make_in_maps(hazards, S, indiv, gene, path, cohort_bank, label, c)
    trace = bool(int(os.environ.get("TRNK_TRACE", "0")))
    res = run_bass_kernel_spmd(nc, in_maps, core_ids=list(range(NCORES)),
                               trace=trace)
    _LAST_RESULTS = res
    outs = np.stack([r["out_vec"][0, :] for r in res.results])  # [8, 4]
    ep = float(outs[:, 0].sum())
    en = float(outs[:, 1].sum())
    other = float(outs[:, 2].mean())
    loss = other - math.log((ep + EPS_COH) / (ep + en + EPS_COH))
    return np.float32(loss)
